# revision 8
# baseline (speedup 1.0000x reference)
"""Trainium2 Bass kernel for nn_GNN_skip_small (gnn_message_passing).

Strategy (8 NeuronCores, SPMD):
  - Row-shard adj and pos across cores (2048 rows each). Host pre-transposes
    each shard to [N, R] so the contraction index lands on SBUF partitions and
    both matmul operands stream in natural layout.
  - adj matmuls run in float32r (f32 bits, fast PE path) for accuracy: the
    infset bits depend on the sign of h1 which is near zero after BatchNorm.
  - pos @ infset runs in bf16 (halves that stream; argmax margin is large).
  - Activations are kept feature-major on device: yT = x_lhsT.T @ adjT_strip.
  - BatchNorm batch stats (sum, sumsq per feature) via a tiny AllReduce.
  - x1 and the infset vector are AllGathered so layer 2 / pos @ infset can run.
  - segment_sum is a one-hot matmul per core; partials are summed on host.
  - Final tiny ops (h_g @ W4, argmax) run on host in f32.
"""

import os
import sys

sys.path.insert(0, "/opt/trn_rl_repo")

import numpy as np
import ml_dtypes

from concourse import bass, bacc, tile, mybir
from concourse.bass_utils import run_bass_kernel_spmd

N = 16384
F = 128          # feature width (= H1 = H2 = 128)
G = 64           # segments
NCORES = 8
R = N // NCORES  # 2048 rows per core
P = 128
KT = N // P      # 128 k tiles over the full row dim
RT = R // P      # 16 row tiles per core
CH = 512         # matmul moving chunk (psum bank width in f32)
NCH = R // CH    # 4 chunks per strip
BN_EPS = 1e-5

F32 = mybir.dt.float32
F32R = mybir.dt.float32r
BF16 = mybir.dt.bfloat16
NP_BF16 = ml_dtypes.bfloat16

_cache = {}


def build_program():
    nc = bacc.Bacc(
        "TRN2", target_bir_lowering=False, debug=False, num_devices=NCORES
    )
    f32, f32r, bf = F32, F32R, BF16

    # ---- per-core external inputs ----
    adjT = nc.declare_dram_parameter("adjT", [N, R], f32r, isOutput=False)
    posT = nc.declare_dram_parameter("posT", [N, R], bf, isOutput=False)
    xin_r = nc.declare_dram_parameter("xin_r", [N, F], f32r, isOutput=False)
    xinT32 = nc.declare_dram_parameter("xinT32", [F, R], f32, isOutput=False)
    xin_seg = nc.declare_dram_parameter("xin_seg", [R, F], f32r, isOutput=False)
    oh32 = nc.declare_dram_parameter("oh32", [R, G], f32r, isOutput=False)
    maskkeep = nc.declare_dram_parameter("maskkeep", [1, R], f32, isOutput=False)
    ones1 = nc.declare_dram_parameter("ones1", [1, P], f32, isOutput=False)
    eye32_d = nc.declare_dram_parameter("eye32", [P, P], f32, isOutput=False)
    eye16_d = nc.declare_dram_parameter("eye16", [P, P], bf, isOutput=False)
    W1a_d = nc.declare_dram_parameter("W1a", [P, P], f32, isOutput=False)
    W1b_d = nc.declare_dram_parameter("W1b", [P, P], f32, isOutput=False)
    W2a_d = nc.declare_dram_parameter("W2a", [P, P], f32, isOutput=False)
    W2b_d = nc.declare_dram_parameter("W2b", [P, P], f32, isOutput=False)
    vec_d = nc.declare_dram_parameter("vecs", [P, 7], f32, isOutput=False)
    # vecs columns: 0=b1, 1=g1, 2=be1, 3=b2, 4=g2, 5=be2, 6=BN_EPS

    # ---- external outputs ----
    seg_o = nc.declare_dram_parameter("seg_o", [G, 3 * F], f32, isOutput=True)
    ovr_o = nc.declare_dram_parameter("ovr_o", [R], f32, isOutput=True)
    inf_o = nc.declare_dram_parameter("inf_o", [R], f32, isOutput=True)

    rg = [list(range(NCORES))]

    with tile.TileContext(nc) as tc:
        with (
            tc.tile_pool(name="const", bufs=1) as const,
            tc.tile_pool(name="strip", bufs=6) as strip_pool,
            tc.tile_pool(name="lhsk", bufs=4) as lhsk_pool,
            tc.tile_pool(name="act", bufs=1) as act_pool,
            tc.tile_pool(name="psum", bufs=1, space="PSUM") as psum,
            tc.tile_pool(name="dram", bufs=1, space="DRAM") as dram,
        ):
            # ---- internal DRAM (collective bounce) ----
            st1_in = dram.tile([P, 2], f32)
            st1_out = dram.tile([P, 2], f32, addr_space="Shared")
            st2_in = dram.tile([P, 2], f32)
            st2_out = dram.tile([P, 2], f32, addr_space="Shared")
            x1loc = dram.tile([R, F], f32r)
            x1full = dram.tile([N, F], f32r, addr_space="Shared")
            infloc = dram.tile([R], bf)
            inffull = dram.tile([N], bf, addr_space="Shared")

            # ---- constants into SBUF ----
            eye32 = const.tile([P, P], f32)
            nc.sync.dma_start(eye32[:], eye32_d[:])
            eye16 = const.tile([P, P], bf)
            nc.sync.dma_start(eye16[:], eye16_d[:])
            W1a = const.tile([P, P], f32)
            nc.sync.dma_start(W1a[:], W1a_d[:])
            W1b = const.tile([P, P], f32)
            nc.sync.dma_start(W1b[:], W1b_d[:])
            W2a = const.tile([P, P], f32)
            nc.sync.dma_start(W2a[:], W2a_d[:])
            W2b = const.tile([P, P], f32)
            nc.sync.dma_start(W2b[:], W2b_d[:])
            vecs = const.tile([P, 7], f32)
            nc.sync.dma_start(vecs[:], vec_d[:])
            ones1_sb = const.tile([1, P], f32)
            nc.sync.dma_start(ones1_sb[:], ones1[:])
            mk = const.tile([1, R], f32)
            nc.sync.dma_start(mk[:], maskkeep[:])
            xinT = const.tile([F, R], f32)
            nc.sync.dma_start(xinT[:], xinT32[:])
            oh32_sb = const.tile([P, RT, G], f32r)
            nc.sync.dma_start(oh32_sb[:], oh32[:].rearrange("(t p) g -> p t g", p=P))
            xinseg_sb = const.tile([P, RT, F], f32r)
            nc.sync.dma_start(
                xinseg_sb[:], xin_seg[:].rearrange("(t p) h -> p t h", p=P)
            )

            # mask broadcast to [P, R] via K=1 matmul: ones1.T @ maskkeep
            mb_ps = psum.tile([P, R], f32, tag="pbig", bufs=1)
            for c in range(NCH):
                nc.tensor.matmul(
                    mb_ps[:, c * CH : (c + 1) * CH],
                    ones1_sb[:],
                    mk[:, c * CH : (c + 1) * CH],
                )
            maskb = const.tile([P, R], f32)
            nc.vector.tensor_copy(maskb[:], mb_ps[:])

            # ---- layer 1: y1T = x_in.T @ adjT (feature-major) ----
            y1_ps = psum.tile([P, R], f32, tag="pbig", bufs=1)
            for k in range(KT):
                xk = lhsk_pool.tile([P, F], f32r, tag="lhsk")
                nc.sync.dma_start(xk[:], xin_r[k * P : (k + 1) * P, :])
                a = strip_pool.tile([P, R], f32r, tag="strip")
                nc.sync.dma_start(a[:], adjT[k * P : (k + 1) * P, :])
                for c in range(NCH):
                    nc.tensor.matmul(
                        y1_ps[:, c * CH : (c + 1) * CH],
                        xk[:],
                        a[:, c * CH : (c + 1) * CH],
                        start=(k == 0),
                        stop=(k == KT - 1),
                    )
            y1T = act_pool.tile([P, R], f32, tag="yT")
            nc.vector.tensor_copy(y1T[:], y1_ps[:])

            # ---- MLP 1 + BN1 stats ----
            z1_ps = psum.tile([P, R], f32, tag="pbig", bufs=1)
            for c in range(NCH):
                sl = slice(c * CH, (c + 1) * CH)
                nc.tensor.matmul(z1_ps[:, sl], W1a[:], xinT[:, sl], start=True, stop=False)
                nc.tensor.matmul(z1_ps[:, sl], W1b[:], y1T[:, sl], start=False, stop=True)
            z1T = act_pool.tile([P, R], f32, tag="zT")
            st1 = const.tile([P, 2], f32)
            nc.scalar.activation(
                z1T[:], z1_ps[:], mybir.ActivationFunctionType.Relu,
                bias=vecs[:, 0:1], accum_out=st1[:, 0:1],
            )
            sq = act_pool.tile([P, R], f32, tag="sq")
            nc.scalar.activation(
                sq[:], z1T[:], mybir.ActivationFunctionType.Square,
                accum_out=st1[:, 1:2],
            )
            nc.sync.dma_start(st1_in[:], st1[:])
            nc.gpsimd.collective_compute(
                "AllReduce", mybir.AluOpType.add, replica_groups=rg,
                ins=[st1_in[:].opt()], outs=[st1_out[:].opt()],
            )
            st1g = const.tile([P, 2], f32)
            nc.sync.dma_start(st1g[:], st1_out[:])

            def bn_scale_bias(stg, g_ap, be_ap, nm):
                mu = const.tile([P, 1], f32, name=f"mu{nm}")
                nc.scalar.mul(mu[:], stg[:, 0:1], 1.0 / N)
                ex2 = const.tile([P, 1], f32, name=f"ex2{nm}")
                nc.scalar.mul(ex2[:], stg[:, 1:2], 1.0 / N)
                musq = const.tile([P, 1], f32, name=f"musq{nm}")
                nc.scalar.square(musq[:], mu[:])
                var = const.tile([P, 1], f32, name=f"var{nm}")
                nc.vector.tensor_sub(var[:], ex2[:], musq[:])
                std = const.tile([P, 1], f32, name=f"std{nm}")
                nc.scalar.activation(
                    std[:], var[:], mybir.ActivationFunctionType.Sqrt,
                    bias=vecs[:, 6:7],
                )
                rstd = const.tile([P, 1], f32, name=f"rstd{nm}")
                nc.vector.reciprocal(rstd[:], std[:])
                s = const.tile([P, 1], f32, name=f"s{nm}")
                nc.vector.tensor_mul(s[:], g_ap, rstd[:])
                tmp = const.tile([P, 1], f32, name=f"tmp{nm}")
                nc.vector.tensor_mul(tmp[:], mu[:], s[:])
                t = const.tile([P, 1], f32, name=f"t{nm}")
                nc.vector.tensor_sub(t[:], be_ap, tmp[:])
                return s, t

            s1, t1 = bn_scale_bias(st1g, vecs[:, 1:2], vecs[:, 2:3], "1")
            x1T = act_pool.tile([P, R], f32, tag="x1T")
            nc.vector.tensor_scalar(
                sq[:], z1T[:], s1[:], t1[:],
                op0=mybir.AluOpType.mult, op1=mybir.AluOpType.add,
            )
            nc.vector.tensor_mul(x1T[:], sq[:], maskb[:])

            # ---- transpose x1 to row-major (f32r), h1, infset ----
            x1rm = act_pool.tile([P, RT, F], f32r, tag="x1rm")
            h1sb = const.tile([P, RT], f32)
            for t in range(RT):
                ps = psum.tile([P, P], f32, tag="pt", bufs=2)
                nc.tensor.transpose(ps[:], x1T[:, t * P : (t + 1) * P], eye32[:])
                nc.vector.tensor_copy(x1rm[:, t, :], ps[:])
                nc.vector.tensor_reduce(
                    h1sb[:, t : t + 1], ps[:], axis=mybir.AxisListType.X,
                    op=mybir.AluOpType.add,
                )
            infsb = const.tile([P, RT], f32)
            nc.vector.tensor_single_scalar(
                infsb[:], h1sb[:], 0.0, op=mybir.AluOpType.is_le
            )
            inf_ps = psum.tile([RT, P], f32, tag="pt", bufs=2)
            nc.tensor.transpose(inf_ps[:], infsb[:], eye32[:])
            inf_row32 = const.tile([RT, P], f32)
            nc.vector.tensor_copy(inf_row32[:], inf_ps[:])
            inf_row16 = const.tile([RT, P], bf)
            nc.vector.tensor_copy(inf_row16[:], inf_ps[:])
            nc.sync.dma_start(inf_o[:].rearrange("(t p) -> t p", p=P), inf_row32[:])
            nc.sync.dma_start(infloc[:].rearrange("(t p) -> t p", p=P), inf_row16[:])
            nc.sync.dma_start(x1loc[:].rearrange("(t p) h -> p t h", p=P), x1rm[:])

            nc.gpsimd.collective_compute(
                "AllGather", mybir.AluOpType.bypass, replica_groups=rg,
                ins=[x1loc[:].opt()], outs=[x1full[:].opt()],
            )
            nc.gpsimd.collective_compute(
                "AllGather", mybir.AluOpType.bypass, replica_groups=rg,
                ins=[infloc[:].opt()], outs=[inffull[:].opt()],
            )

            # infset lhsT: A[a, b] = inf[a*128+b]; AT[p, j] = inf[j*128+p]
            Asb = const.tile([P, P], bf)
            nc.sync.dma_start(Asb[:], inffull[:].rearrange("(a b) -> a b", a=P))
            at_ps = psum.tile([P, P], bf, tag="ptbf", bufs=1)
            nc.tensor.transpose(at_ps[:], Asb[:], eye16[:])
            ATsb = const.tile([P, P], bf)
            nc.vector.tensor_copy(ATsb[:], at_ps[:])

            # ---- layer 2: y2T = x1.T @ adjT ----
            y2_ps = psum.tile([P, R], f32, tag="pbig", bufs=1)
            for k in range(KT):
                x1k = lhsk_pool.tile([P, F], f32r, tag="lhsk")
                nc.sync.dma_start(x1k[:], x1full[k * P : (k + 1) * P, :])
                a2 = strip_pool.tile([P, R], f32r, tag="strip")
                nc.sync.dma_start(a2[:], adjT[k * P : (k + 1) * P, :])
                for c in range(NCH):
                    nc.tensor.matmul(
                        y2_ps[:, c * CH : (c + 1) * CH],
                        x1k[:],
                        a2[:, c * CH : (c + 1) * CH],
                        start=(k == 0),
                        stop=(k == KT - 1),
                    )
            y2T = act_pool.tile([P, R], f32, tag="yT")
            nc.vector.tensor_copy(y2T[:], y2_ps[:])

            # ---- MLP 2 + BN2 ----
            z2_ps = psum.tile([P, R], f32, tag="pbig", bufs=1)
            for c in range(NCH):
                sl = slice(c * CH, (c + 1) * CH)
                nc.tensor.matmul(z2_ps[:, sl], W2a[:], x1T[:, sl], start=True, stop=False)
                nc.tensor.matmul(z2_ps[:, sl], W2b[:], y2T[:, sl], start=False, stop=True)
            z2T = act_pool.tile([P, R], f32, tag="zT")
            st2 = const.tile([P, 2], f32)
            nc.scalar.activation(
                z2T[:], z2_ps[:], mybir.ActivationFunctionType.Relu,
                bias=vecs[:, 3:4], accum_out=st2[:, 0:1],
            )
            sq2 = act_pool.tile([P, R], f32, tag="sq")
            nc.scalar.activation(
                sq2[:], z2T[:], mybir.ActivationFunctionType.Square,
                accum_out=st2[:, 1:2],
            )
            nc.sync.dma_start(st2_in[:], st2[:])
            nc.gpsimd.collective_compute(
                "AllReduce", mybir.AluOpType.add, replica_groups=rg,
                ins=[st2_in[:].opt()], outs=[st2_out[:].opt()],
            )
            st2g = const.tile([P, 2], f32)
            nc.sync.dma_start(st2g[:], st2_out[:])
            s2, t2 = bn_scale_bias(st2g, vecs[:, 4:5], vecs[:, 5:6], "2")
            x2T = act_pool.tile([P, R], f32, tag="x1T2")
            nc.vector.tensor_scalar(
                sq2[:], z2T[:], s2[:], t2[:],
                op0=mybir.AluOpType.mult, op1=mybir.AluOpType.add,
            )
            nc.vector.tensor_mul(x2T[:], sq2[:], maskb[:])

            x2rm = act_pool.tile([P, RT, F], f32r, tag="x2rm")
            for t in range(RT):
                ps2 = psum.tile([P, P], f32, tag="pt", bufs=2)
                nc.tensor.transpose(ps2[:], x2T[:, t * P : (t + 1) * P], eye32[:])
                nc.vector.tensor_copy(x2rm[:, t, :], ps2[:])

            # ---- segment sum: onehot.T @ [x_in | x1 | x2] ----
            seg_ps = psum.tile([G, 3 * F], f32, tag="pseg", bufs=1)
            for t in range(RT):
                st_, sp_ = (t == 0), (t == RT - 1)
                nc.tensor.matmul(seg_ps[:, 0:F], oh32_sb[:, t, :],
                                 xinseg_sb[:, t, :], start=st_, stop=sp_)
            for t in range(RT):
                st_, sp_ = (t == 0), (t == RT - 1)
                nc.tensor.matmul(seg_ps[:, F : 2 * F], oh32_sb[:, t, :],
                                 x1rm[:, t, :], start=st_, stop=sp_)
            for t in range(RT):
                st_, sp_ = (t == 0), (t == RT - 1)
                nc.tensor.matmul(seg_ps[:, 2 * F : 3 * F], oh32_sb[:, t, :],
                                 x2rm[:, t, :], start=st_, stop=sp_)
            seg_sb = const.tile([G, 3 * F], f32)
            nc.vector.tensor_copy(seg_sb[:], seg_ps[:])
            nc.sync.dma_start(seg_o[:], seg_sb[:])

            # ---- ovr = (pos @ infset) local rows, inf stationary ----
            ovr_ps = psum.tile([1, R], f32, tag="pbig", bufs=1)
            for k in range(KT):
                pstrip = strip_pool.tile([P, R], bf, tag="strip")
                nc.sync.dma_start(pstrip[:], posT[k * P : (k + 1) * P, :])
                for c in range(NCH):
                    nc.tensor.matmul(
                        ovr_ps[0:1, c * CH : (c + 1) * CH],
                        ATsb[:, k : k + 1],
                        pstrip[:, c * CH : (c + 1) * CH],
                        start=(k == 0),
                        stop=(k == KT - 1),
                    )
            ovr_sb = const.tile([1, R], f32)
            nc.vector.tensor_copy(ovr_sb[:], ovr_ps[:])
            nc.sync.dma_start(ovr_o[:].rearrange("(a r) -> a r", a=1), ovr_sb[:])

    nc.compile()
    return nc


def _prep_inputs(adj, x_in, mask, pos, idx, W1, b1, W2, b2, g1, be1, g2, be2):
    """Host-side shard prep. Returns list of per-core input dicts."""
    adj = np.asarray(adj, dtype=np.float32)
    pos = np.asarray(pos, dtype=np.float32)
    x_in = np.ascontiguousarray(np.asarray(x_in, dtype=np.float32))
    maskf = np.asarray(mask).astype(bool)
    idx = np.asarray(idx).astype(np.int64)

    onehot = (idx[:, None] == np.arange(G)[None, :])
    oh32_full = onehot.astype(np.float32)
    keep_full = (~maskf).astype(np.float32)

    eye32 = np.eye(P, dtype=np.float32)
    eye16 = np.eye(P, dtype=NP_BF16)
    ones1 = np.ones((1, P), dtype=np.float32)
    W1 = np.asarray(W1, np.float32)
    W2 = np.asarray(W2, np.float32)
    vecs = np.stack(
        [np.asarray(v, np.float32) for v in (b1, g1, be1, b2, g2, be2)]
        + [np.full(P, BN_EPS, np.float32)],
        axis=1,
    )  # [128, 7]
    common = {
        "xin_r": x_in,
        "eye32": eye32,
        "eye16": eye16,
        "ones1": ones1,
        "W1a": np.ascontiguousarray(W1[:P]),
        "W1b": np.ascontiguousarray(W1[P:]),
        "W2a": np.ascontiguousarray(W2[:P]),
        "W2b": np.ascontiguousarray(W2[P:]),
        "vecs": np.ascontiguousarray(vecs),
    }
    pos_bf = pos.astype(NP_BF16)
    in_maps = []
    for c in range(NCORES):
        rs = slice(c * R, (c + 1) * R)
        m = dict(common)
        m["adjT"] = np.ascontiguousarray(adj[rs].T)
        m["posT"] = np.ascontiguousarray(pos_bf[rs].T)
        m["xinT32"] = np.ascontiguousarray(x_in[rs].T)
        m["xin_seg"] = np.ascontiguousarray(x_in[rs])
        m["oh32"] = np.ascontiguousarray(oh32_full[rs])
        m["maskkeep"] = np.ascontiguousarray(keep_full[rs][None, :])
        in_maps.append(m)
    return in_maps


def _run_traced(nc, in_maps):
    """Run via PJRT with the axon NTFF profiling hook; print HW exec time and
    write a local profile json. Falls back to an untraced run on any error."""
    import glob
    import tempfile

    from concourse import bass2jax

    sys.path.insert(0, "/root/.axon_site")
    try:
        from trn_agent_boot.trn_boot import _ntff_profile_via_ctypes

        hookf = _ntff_profile_via_ctypes("/opt/axon/libaxon_pjrt.so")
        assert hookf is not None
    except Exception as e:
        print(f"profiling unavailable ({e}); running untraced", flush=True)
        return bass2jax.run_bass_via_pjrt(nc, in_maps, n_cores=NCORES)

    outdir = os.environ.get("KERNEL_TRACE_DIR") or tempfile.mkdtemp(prefix="ntff_")
    os.makedirs(outdir, exist_ok=True)
    with hookf(outdir, [0]):
        results = bass2jax.run_bass_via_pjrt(nc, in_maps, n_cores=NCORES)

    try:
        ntffs = sorted(glob.glob(os.path.join(outdir, "*_body*.ntff")))
        neffs = sorted(
            glob.glob(os.path.join(outdir, "*.neff")),
            key=os.path.getsize, reverse=True,
        )
        print(f"profile dir {outdir}: ntffs={ntffs} neffs={neffs}", flush=True)
        if not ntffs or not neffs:
            return results
        import subprocess

        jpath = os.path.join(outdir, "prof.json")
        subprocess.check_call(
            ["neuron-profile", "view", "--ignore-nc-buf-usage",
             "-s", os.path.basename(ntffs[0]), "-n", os.path.basename(neffs[0]),
             "--output-format=json", "--output-file=prof.json"],
            cwd=outdir, env={**os.environ, "NEURON_PROFILE_DBG_OUTPUT": "2"},
        )
        import json

        d = json.load(open(jpath))
        insts = d["instruction"]
        t0 = min(i["timestamp"] for i in insts)
        t1 = max(i["timestamp"] + i["duration"] for i in insts)
        exec_time_ns = t1 - t0
        _cache["exec_time_ns"] = exec_time_ns
        _cache["prof_json"] = jpath
        print(f"HW exec time: {exec_time_ns} ns", flush=True)
    except Exception as e:
        print(f"trace processing failed: {type(e).__name__}: {e}", flush=True)
    return results


def kernel(adj, x_in, mask, pos, idx, seed_set, step,
           W1, b1, W2, b2, W4, b4, g1, be1, g2, be2):
    if "nc" not in _cache:
        _cache["nc"] = build_program()
    nc = _cache["nc"]

    in_maps = _prep_inputs(adj, x_in, mask, pos, idx, W1, b1, W2, b2, g1, be1, g2, be2)

    trace = bool(int(os.environ.get("KERNEL_TRACE", "0")))
    from concourse import bass2jax

    if trace:
        outs = _run_traced(nc, in_maps)
    else:
        outs = bass2jax.run_bass_via_pjrt(nc, in_maps, n_cores=NCORES)

    seg_total = np.zeros((G, 3 * F), np.float64)
    ovr = np.empty(N, np.float32)
    inf = np.empty(N, np.float32)
    for c in range(NCORES):
        seg_total += np.asarray(outs[c]["seg_o"], np.float64)
        ovr[c * R : (c + 1) * R] = outs[c]["ovr_o"]
        inf[c * R : (c + 1) * R] = outs[c]["inf_o"]

    W4 = np.asarray(W4, np.float32)
    b4 = np.asarray(b4, np.float32)
    out = np.maximum(seg_total.astype(np.float32) @ W4 + b4, 0.0)  # [G, 1]
    infset = inf[:, None].astype(np.float32)                       # [N, 1]
    ovr_m = ovr.copy()
    ovr_m[np.asarray(seed_set).astype(np.int64)] = -np.inf
    new_seed = np.int32(np.argmax(ovr_m))
    return out, infset, new_seed
